# revision 66
# baseline (speedup 1.0000x reference)
"""GIN-style 3-layer GNN encoder on 8 Trainium2 NeuronCores (Bass/Tile).

Reference computation (fp32):
    h = x @ W_in.T + b_in                                  [50000, 96]
    for l in 0..2:
        agg = segment_sum(h[src], dst, N)                  [50000, 96]
        h = (h + agg) @ W_layers[l].T + b_layers[l]
    out = concat([h0..h3], 1) @ W_out.T + b_out            [50000, 128]

The layers are linear, so the per-layer weight matmuls commute past the
aggregations:  out = sum_k g_k C_k + r  with g_k = (I+A)^k h0,
C_k = (W1^T..Wk^T) Wout_k^T and r a rank-3 per-node bias correction from
(1, 1+d, 1+2d+Ad) — all host-precomputed.  On device each layer is then
just g_{k+1} = g_k + A g_k, which shortens the window -> AllGather path.

Distribution: nodes are partitioned across the 8 cores (6250/core) via a
host-side permutation; each edge is owned by the core that owns its dst
node.  Each layer the updated features are AllGathered into per-layer
replicated row-major fp16 tables h_fullA/h_fullB (Shared DRAM; the A/B
split is window-aligned at 3200 nodes so AG_A fires mid-layer and hides
under the B-half compute, while AG_B's latency is covered by
front-loading the next layer's A-class gathers; 8*HALF keeps gather
indices int16-safe).

Per-core segment sum: a core's node range is split into 49 windows of
128 nodes with per-window tile counts T_a_w/T_b_w (shared across cores;
the permutation greedily minimizes sum_w max_core ceil(count/128), ~7%
padding).  Edge features are fetched with gpsimd dma_gather (fp16 256B
rows, 1024 idxs per instruction = one 64-descriptor packet per SDMA
engine — the hardware maximum — round-robin over the 4 SWDGE queues,
num_idxs registers hoisted).  The Q7 descriptor generation (~1us fixed +
~2ns/idx) is the overall bottleneck, so everything else is scheduled to
hide under it.  For each window the one-hot
onehot[e, t, j] = (dst_local[e, t] == j) is built on DVE with one
broadcast is_equal, and the PE accumulates
    psum[96, 128] += gathered_tile[128e, 96].T @ onehot_tile[128e, 128]
which is aggT for the window; a DVE add writes g_{k+1} = g_k + agg
directly, PE transposes stream the row-major shard, and the two
AllGathers fire as soon as their half of the windows is transposed.
A tiny warm-up collective absorbs the NRT first-collective barrier.
"""
import sys

sys.path.insert(0, "/opt/trn_rl_repo")

import numpy as np

N_NODES = 50000
N_EDGES = 800000
IN_DIM = 128
HID = 96
OUT_DIM = 128
N_LAYERS = 3
N_CORES = 8
NPC = N_NODES // N_CORES          # 6250 nodes per core
WIN = 128                         # window width (nodes)
NW = (NPC + WIN - 1) // WIN       # 49 windows per core (last = 106 nodes)
HALF = 3200                       # per-core A/B split, window-aligned
CLS = N_CORES * HALF              # 31744: A-class size
AW = HALF // WIN                  # 31 full-A windows per core
REM_A = HALF - AW * WIN           # 0: split is window-aligned
CHUNK_W = 4                       # windows per gather buffer
GT = 8                            # tiles per dma_gather (1024 idxs)
CW_N = 512                        # node-chunk for dense matmuls

_cache = {}


def _balance_nodes(src0, dst0):
    """Permute node ids to minimize total gather tiles.

    A node's A/B class (which replicated gather table its row lives in) is
    frozen to its OLD id (< CLS -> A).  The greedy places heavy nodes
    first, charging each placement the number of 128-edge tiles it would
    ADD to the per-window profile T_x[w] = max_c ceil(load_x[c,w]/128),
    so the final per-window tile counts (shared across cores, compile-time
    loop bounds) carry minimal padding.  Returns perm (old id -> new id).
    """
    deg_a = np.bincount(dst0[src0 < CLS], minlength=N_NODES).astype(np.int64)
    deg_b = np.bincount(dst0[src0 >= CLS], minlength=N_NODES).astype(np.int64)
    nbins = N_CORES * NW
    base = np.empty(nbins, np.int64)
    cap = np.empty(nbins, np.int64)
    w_of = np.empty(nbins, np.int64)
    for b in range(nbins):
        c, w = divmod(b, NW)
        base[b] = c * NPC + w * WIN
        cap[b] = min(WIN, NPC - w * WIN)
        w_of[b] = w
    woff = base % NPC
    q_a = np.clip(HALF - woff, 0, cap)   # A slots = first q_a of the window
    q_b = cap - q_a

    mu_a = max(1.0, deg_a.sum() / nbins)
    mu_b = max(1.0, deg_b.sum() / nbins)
    order = np.argsort(-(deg_a + deg_b), kind="stable")
    a_load = np.zeros(nbins, np.int64)
    b_load = np.zeros(nbins, np.int64)
    tmax_a = np.zeros(NW, np.int64)      # current per-window tile profile
    tmax_b = np.zeros(NW, np.int64)
    a_left = q_a.copy()
    b_left = q_b.copy()
    a_pos = np.zeros(nbins, np.int64)
    b_pos = q_a.copy()
    perm = np.empty(N_NODES, np.int64)
    BIG = 1e7
    for n in order:
        na = a_load + deg_a[n]
        nb = b_load + deg_b[n]
        add = (np.maximum(0, -(-na // 128) - tmax_a[w_of])
               + np.maximum(0, -(-nb // 128) - tmax_b[w_of]))
        # the tmax bias spreads profile growth across windows (flat
        # profiles keep per-chunk gather buffers small and uniform)
        phi = (add * BIG + np.maximum(na / mu_a, nb / mu_b)
               + (tmax_a[w_of] + tmax_b[w_of]) * 50.0)
        if n < CLS:
            phi = np.where(a_left > 0, phi, np.inf)
            b_ = int(np.argmin(phi))
            perm[n] = base[b_] + a_pos[b_]
            a_pos[b_] += 1
            a_left[b_] -= 1
        else:
            phi = np.where(b_left > 0, phi, np.inf)
            b_ = int(np.argmin(phi))
            perm[n] = base[b_] + b_pos[b_]
            b_pos[b_] += 1
            b_left[b_] -= 1
        a_load[b_] += deg_a[n]
        b_load[b_] += deg_b[n]
        w_ = w_of[b_]
        tmax_a[w_] = max(tmax_a[w_], -(-a_load[b_] // 128))
        tmax_b[w_] = max(tmax_b[w_], -(-b_load[b_] // 128))
    return perm


def _prep(edge_index):
    """Host-side edge bucketing -> per-core gather index / dst tables.

    Tile counts are per-window (T_a_w, T_b_w — shared across cores), so
    padding is only the across-core ceil gap the balancer leaves."""
    src0 = edge_index[0].astype(np.int64)
    dst0 = edge_index[1].astype(np.int64)
    perm = _balance_nodes(src0, dst0)
    src = perm[src0]
    dst = perm[dst0]
    core = dst // NPC
    din = dst % NPC
    w = din // WIN
    dstl = din % WIN
    s_in = src % NPC
    c_src = src // NPC
    is_b = (s_in >= HALF).astype(np.int64)
    pos = np.where(is_b == 0, c_src * HALF + s_in,
                   c_src * (NPC - HALF) + s_in - HALF)  # int16-safe

    key = (core * NW + w) * 2 + is_b
    order = np.argsort(key, kind="stable")
    s_pos = pos[order]
    s_dstl = dstl[order]
    s_key = key[order]
    s_b = is_b[order]

    counts = np.bincount(key, minlength=N_CORES * NW * 2)
    cnt_cw = counts.reshape(N_CORES, NW, 2)
    T_a_w = np.maximum(1, -(-cnt_cw[:, :, 0].max(axis=0) // 128))  # [NW]
    T_b_w = np.maximum(1, -(-cnt_cw[:, :, 1].max(axis=0) // 128))
    offs_a = np.zeros(NW + 1, np.int64)
    offs_a[1:] = np.cumsum(T_a_w)
    offs_b = np.zeros(NW + 1, np.int64)
    offs_b[1:] = np.cumsum(T_b_w)
    toff = np.zeros(NW + 1, np.int64)
    toff[1:] = np.cumsum(T_a_w + T_b_w)
    TA, TB = int(offs_a[-1]), int(offs_b[-1])
    TT = int(toff[-1])

    starts = np.zeros(N_CORES * NW * 2, np.int64)
    starts[1:] = np.cumsum(counts)[:-1]
    rank = np.arange(len(s_key)) - starts[s_key]

    c_arr = s_key // (2 * NW)
    w_arr = (s_key // 2) % NW

    TA8 = -(-TA // 8) * 8   # fetch ranges are 8-tile aligned
    TB8 = -(-TB // 8) * 8
    idx_a = np.zeros((N_CORES, TA8 * 128), np.int16)
    idx_b = np.zeros((N_CORES, TB8 * 128), np.int16)
    dstl_arr = np.full((N_CORES, TT, 128), -1.0, np.float16)

    a_m = s_b == 0
    flat = c_arr[a_m] * (TA8 * 128) + offs_a[w_arr[a_m]] * 128 + rank[a_m]
    idx_a.reshape(-1)[flat] = s_pos[a_m].astype(np.int16)
    flat = ((c_arr[a_m] * TT + toff[w_arr[a_m]] + rank[a_m] // 128) * 128
            + rank[a_m] % 128)
    dstl_arr.reshape(-1)[flat] = s_dstl[a_m].astype(np.float16)

    b_m = ~a_m
    flat = c_arr[b_m] * (TB8 * 128) + offs_b[w_arr[b_m]] * 128 + rank[b_m]
    idx_b.reshape(-1)[flat] = s_pos[b_m].astype(np.int16)
    t_g = T_a_w[w_arr[b_m]] + rank[b_m] // 128
    flat = (c_arr[b_m] * TT + toff[w_arr[b_m]] + t_g) * 128 + rank[b_m] % 128
    dstl_arr.reshape(-1)[flat] = s_dstl[b_m].astype(np.float16)

    def wrap(vals):  # [T*128] -> [128, T*8] int16 wrapped+replicated
        v = vals.reshape(-1, 16).T
        return np.tile(v, (8, 1)).copy()

    idx_a_w = np.stack([wrap(idx_a[c]) for c in range(N_CORES)])
    idx_b_w = np.stack([wrap(idx_b[c]) for c in range(N_CORES)])
    dstloc = np.ascontiguousarray(dstl_arr.transpose(0, 2, 1))  # [C,128,TT]
    return (idx_a_w, idx_b_w, dstloc, tuple(int(t) for t in T_a_w),
            tuple(int(t) for t in T_b_w), perm)


def _build(T_a_w, T_b_w):
    from concourse import bacc, tile, mybir, library_config

    dt = mybir.dt
    offs_a = np.zeros(NW + 1, np.int64)
    offs_a[1:] = np.cumsum(T_a_w)
    offs_b = np.zeros(NW + 1, np.int64)
    offs_b[1:] = np.cumsum(T_b_w)
    toff = np.zeros(NW + 1, np.int64)
    toff[1:] = np.cumsum(np.add(T_a_w, T_b_w))
    TA, TB, TT = int(offs_a[-1]), int(offs_b[-1]), int(toff[-1])
    TA8 = -(-TA // 8) * 8
    TB8 = -(-TB // 8) * 8
    nc = bacc.Bacc("TRN2", target_bir_lowering=False, debug=False,
                   num_devices=N_CORES, num_swdge_queues=4)

    # ---- I/O ----
    xT_in = nc.dram_tensor("xT", [IN_DIM, NPC], dt.float32, kind="ExternalInput")
    w_inT_in = nc.dram_tensor("w_inT", [IN_DIM, HID], dt.float32,
                              kind="ExternalInput")
    b_in_in = nc.dram_tensor("b_in", [HID, 1], dt.float32, kind="ExternalInput")
    w_out4_in = nc.dram_tensor("w_out4", [N_LAYERS + 1, HID, OUT_DIM],
                               dt.float16, kind="ExternalInput")
    rcorrT_in = nc.dram_tensor("rcorrT", [OUT_DIM, NPC], dt.float16,
                               kind="ExternalInput")
    iota_in = nc.dram_tensor("iota", [128, WIN], dt.float16,
                             kind="ExternalInput")
    id96_in = nc.dram_tensor("id96", [HID, HID], dt.float16,
                             kind="ExternalInput")
    id128_in = nc.dram_tensor("id128", [128, 128], dt.float32,
                              kind="ExternalInput")
    idx_a_in = nc.dram_tensor("idx_a", [128, TA8 * 8], dt.int16,
                              kind="ExternalInput")
    idx_b_in = nc.dram_tensor("idx_b", [128, TB8 * 8], dt.int16,
                              kind="ExternalInput")
    dstloc_in = nc.dram_tensor("dstloc", [128, TT], dt.float16,
                               kind="ExternalInput")
    out_ext = nc.dram_tensor("out", [NPC, OUT_DIM], dt.float32,
                             kind="ExternalOutput")

    f32, f32r, f16 = dt.float32, dt.float32r, dt.float16

    with tile.TileContext(nc, num_cores=N_CORES) as tc:
        nc.gpsimd.load_library(library_config.mlp)
        with tc.tile_pool(name="persist", bufs=1) as pp, \
             tc.tile_pool(name="xpool", bufs=5) as xpool, \
             tc.tile_pool(name="ga", bufs=5) as ga_pool, \
             tc.tile_pool(name="gb", bufs=3) as gb_pool, \
             tc.tile_pool(name="oh", bufs=2) as oh_pool, \
             tc.tile_pool(name="otile", bufs=2) as ot_pool, \
             tc.tile_pool(name="ps_agg", bufs=4, space="PSUM") as ps_agg, \
             tc.tile_pool(name="ps_big", bufs=2, space="PSUM") as ps_big, \
             tc.tile_pool(name="ps_tr", bufs=2, space="PSUM") as ps_tr, \
             tc.tile_pool(name="dram", bufs=1, space="DRAM") as dram:

            def load(name, shape, dtype, src_ap):
                t = pp.tile(shape, dtype, name=name)
                nc.sync.dma_start(out=t[:], in_=src_ap)
                return t

            w_inT = load("w_inT", [IN_DIM, HID], f32r, w_inT_in[:].bitcast(f32r))
            b_in = load("b_in", [HID, 1], f32, b_in_in[:])
            iota = load("iota", [128, WIN], f16, iota_in[:])
            id96 = load("id96", [HID, HID], f16, id96_in[:])
            id128 = load("id128", [128, 128], f32, id128_in[:])

            # ping-pong feature states: h_state[s % 2] holds g_s; the
            # output projection accumulates per layer into out_acc, so
            # older states are dead once their layer + accumulation ran.
            h_state = [pp.tile([HID, NPC], f16, name=f"h{s}")
                       for s in range(2)]
            out_acc = pp.tile([OUT_DIM, NPC], f16, name="out_acc")
            rm_buf = pp.tile([128, NW, 128], f16, name="rm_buf")

            # One table pair per layer: Shared DRAM requires a single
            # writer, and separate tables remove WAR hazards between a
            # layer's gathers and the next AllGather.
            h_fullA = [dram.tile([CLS, 128], f16, addr_space="Shared",
                                 name=f"h_fullA{s}")
                       for s in range(N_LAYERS)]
            h_fullB = [dram.tile([N_NODES - CLS, 128], f16,
                                 addr_space="Shared", name=f"h_fullB{s}")
                       for s in range(N_LAYERS)]
            bounceA = dram.tile([HALF, 128], f16)
            bounceB = dram.tile([NPC - HALF, 128], f16)

            node_chunks = [(j * CW_N, min(CW_N, NPC - j * CW_N))
                           for j in range(-(-NPC // CW_N))]

            def transpose_windows(s, w0, w1):
                for t in range(w0, w1):
                    n0 = t * 128
                    tn = min(128, NPC - n0)
                    pst = ps_tr.tile([128, HID], f16, name="pst")
                    nc.tensor.transpose(pst[:tn, :],
                                        h_state[s % 2][:, n0:n0 + tn],
                                        id96[:])
                    nc.scalar.copy(rm_buf[:tn, t, 0:HID], pst[:tn, :])

            def bounce_ag_a(s):
                nc.sync.dma_start(
                    out=bounceA[0:AW * 128, :].rearrange(
                        "(t p) d -> p t d", p=128),
                    in_=rm_buf[:, 0:AW, :])
                nc.gpsimd.collective_compute(
                    "AllGather", mybir.AluOpType.bypass,
                    ins=[bounceA.opt()], outs=[h_fullA[s].opt()],
                    replica_groups=[list(range(N_CORES))])

            def bounce_ag_b(s):
                nb_full = NW - AW - 1   # full windows AW .. NW-2
                nc.sync.dma_start(
                    out=bounceB[0:nb_full * 128, :].rearrange(
                        "(t p) d -> p t d", p=128),
                    in_=rm_buf[:, AW:NW - 1, :])
                o1 = nb_full * 128
                last_n = NPC - (NW - 1) * WIN
                nc.sync.dma_start(out=bounceB[o1:o1 + last_n, :],
                                  in_=rm_buf[0:last_n, NW - 1, :])
                nc.gpsimd.collective_compute(
                    "AllGather", mybir.AluOpType.bypass,
                    ins=[bounceB.opt()], outs=[h_fullB[s].opt()],
                    replica_groups=[list(range(N_CORES))])

            def out_proj(n0, cw):
                ps = ps_big.tile([OUT_DIM, CW_N], f32, name="pso", tag="psb")
                nc.tensor.matmul(ps[:, :cw], w_out4[N_LAYERS][:],
                                 h_state[N_LAYERS % 2][:, n0:n0 + cw],
                                 start=True, stop=True)
                ot = ot_pool.tile([OUT_DIM, CW_N], f32, name="ot")
                nc.vector.tensor_tensor(ot[:, :cw], ps[:, :cw],
                                        out_acc[:, n0:n0 + cw],
                                        mybir.AluOpType.add)
                for tt in range(-(-cw // 128)):
                    t0 = tt * 128
                    tn = min(128, cw - t0)
                    pst = ps_tr.tile([128, 128], f32, name="psto", tag="pst")
                    nc.tensor.transpose(pst[:tn, :], ot[:, t0:t0 + tn],
                                        id128[:])
                    orow = ot_pool.tile([128, 128], f32, name="orow")
                    nc.scalar.copy(orow[:tn, :], pst[:tn, :])
                    nc.sync.dma_start(
                        out=out_ext[n0 + t0:n0 + t0 + tn, :],
                        in_=orow[:tn, :])

            PH_A_END = AW * WIN   # 3968: phase-A node frontier

            def make_advance(s):
                """Incrementally issue dense matmul / transposes / AGs for
                h_state[s] as the node frontier moves.  AG_A fires as soon
                as window AW is transposed (mid-layer), AG_B at the end, so
                each collective overlaps the other half's compute."""
                st = {"dense": 0, "trans": 0}

                def advance(frontier):
                    frontier = min(frontier, NPC)
                    if s == 0 or s == N_LAYERS:
                        while st["dense"] < frontier:
                            n0 = st["dense"]
                            lim = PH_A_END if n0 < PH_A_END else NPC
                            cw = min(CW_N, frontier - n0, lim - n0)
                            if s == 0:
                                xb = xpool.tile([IN_DIM, CW_N], f32r,
                                                name="xb")
                                nc.sync.dma_start(
                                    out=xb[:, :cw],
                                    in_=xT_in[:, n0:n0 + cw].bitcast(f32r))
                                ps = ps_big.tile([HID, CW_N], f32, name="psb")
                                nc.tensor.matmul(ps[:, :cw], w_inT[:],
                                                 xb[:, :cw],
                                                 start=True, stop=True)
                                nc.scalar.add(h_state[0][:, n0:n0 + cw],
                                              ps[:, :cw], b_in[:])
                            else:
                                out_proj(n0, cw)
                            st["dense"] += cw
                    else:
                        # h_state[s] is written directly by the window
                        # aggregation adds; just move the frontier.
                        st["dense"] = max(st["dense"], frontier)
                    if s < N_LAYERS:
                        while st["trans"] < NW and \
                                min((st["trans"] + 1) * WIN, NPC) <= st["dense"]:
                            transpose_windows(s, st["trans"], st["trans"] + 1)
                            w_done = st["trans"]
                            st["trans"] += 1
                            if w_done == AW - 1:
                                bounce_ag_a(s)
                            elif w_done == NW - 1:
                                bounce_ag_b(s)
                return advance

            qrr = [0]
            nreg = {}   # hoisted num_idxs registers (avoid a MOVE per gather)

            def emit_gathers(gbuf, src_view, idx_tile, base_tile, n_tiles):
                for s0 in range(0, n_tiles, GT):
                    sn = min(GT, n_tiles - s0)
                    v = sn * 128
                    if v not in nreg:
                        nreg[v] = nc.gpsimd.to_reg(v)
                    nc.gpsimd.dma_gather(
                        gbuf[:, s0:s0 + sn, :], src_view,
                        idx_tile[:, (base_tile + s0) * 8:
                                 (base_tile + s0 + sn) * 8],
                        num_idxs=v, num_idxs_reg=nreg[v],
                        elem_size=128, single_packet=True,
                        queue_num=qrr[0] % 4)
                    qrr[0] += 1

            # Tiny warm-up collective: absorbs the NRT first-collective
            # barrier (~30us) under the input loads / projection.  (A
            # dependency-free variant that dispatches immediately measured
            # consistently WORSE — the barrier rendezvous runs long when
            # entered before the cores finish setup — so the short
            # memset->DMA chain before dispatch is deliberate.)
            warm_src = dram.tile([1, 128], f16)
            warm_dst = dram.tile([N_CORES, 128], f16, addr_space="Shared")
            warm_sb = pp.tile([1, 128], f16, name="warm_sb")
            nc.vector.memset(warm_sb[:], 0.0)
            nc.sync.dma_start(out=warm_src[:], in_=warm_sb[:])
            nc.gpsimd.collective_compute(
                "AllGather", mybir.AluOpType.bypass,
                ins=[warm_src.opt()], outs=[warm_dst.opt()],
                replica_groups=[list(range(N_CORES))])

            # ---- input projection (phase-split epilogue) ----
            adv0 = make_advance(0)
            adv0(PH_A_END)
            # bulk tables load after the phase-A projection's x chunks so
            # the first AllGather isn't queued behind them
            idx_a = load("idx_a", [128, TA8 * 8], dt.int16, idx_a_in[:])
            idx_b = load("idx_b", [128, TB8 * 8], dt.int16, idx_b_in[:])
            dstloc = load("dstloc", [128, TT], f16, dstloc_in[:])
            w_out4 = [load(f"w_out4_{s}", [HID, OUT_DIM], f16, w_out4_in[s])
                      for s in range(N_LAYERS + 1)]
            rcorrT = load("rcorrT", [OUT_DIM, NPC], f16, rcorrT_in[:])
            adv0(NPC)

            # ---- GIN layers ----
            w_chunks = [(c0, min(CHUNK_W, NW - c0))
                        for c0 in range(0, NW, CHUNK_W)]
            # Instruction-aligned fetch ranges: chunk buffer c fetches
            # global tiles [F[c], F[c+1]), F multiples of 8 tiles, so every
            # dma_gather is a full 1024-idx instruction (the tail spills a
            # few of the next chunk's tiles into this buffer; boundary
            # windows read from two buffers).
            def fetch_bounds(offs):
                F = [0]
                for c0, cw in w_chunks:
                    F.append(int(-(-int(offs[c0 + cw]) // 8) * 8))
                return F
            F_a = fetch_bounds(offs_a)
            F_b = fetch_bounds(offs_b)
            GA_MAX = max(F_a[i + 1] - F_a[i] for i in range(len(w_chunks)))
            GB_MAX = max(F_b[i + 1] - F_b[i] for i in range(len(w_chunks)))
            OH_MAX = max(T_a_w[w] + T_b_w[w] for w in range(NW))
            def accumulate_out(l):
                # out_acc += C_l @ g_l (g_l complete since last layer);
                # l == 0 initializes with the rank-3 bias correction.
                for n0, cwn in node_chunks:
                    ps = ps_big.tile([OUT_DIM, CW_N], f32, name="psA",
                                     tag="psb")
                    nc.tensor.matmul(ps[:, :cwn], w_out4[l][:],
                                     h_state[l % 2][:, n0:n0 + cwn],
                                     start=True, stop=True)
                    if l == 0:
                        nc.vector.tensor_tensor(
                            out_acc[:, n0:n0 + cwn], ps[:, :cwn],
                            rcorrT[:, n0:n0 + cwn], mybir.AluOpType.add)
                    else:
                        nc.vector.tensor_tensor(
                            out_acc[:, n0:n0 + cwn], ps[:, :cwn],
                            out_acc[:, n0:n0 + cwn], mybir.AluOpType.add)

            for l in range(N_LAYERS):
                adv = make_advance(l + 1)
                ga_tiles = {}
                gb_tiles = {}
                accumulate_out(l)

                def issue_a(ci, l=l):
                    nt = F_a[ci + 1] - F_a[ci]
                    g = ga_pool.tile([128, GA_MAX, 128], f16, name="g_a")
                    emit_gathers(g, h_fullA[l][:], idx_a, F_a[ci], nt)
                    ga_tiles[ci] = g

                def issue_b(ci, l=l):
                    nt = F_b[ci + 1] - F_b[ci]
                    g = gb_pool.tile([128, GB_MAX, 128], f16, name="g_b")
                    emit_gathers(g, h_fullB[l][:], idx_b, F_b[ci], nt)
                    gb_tiles[ci] = g

                # A-class gathers only need AG_A (already landed mid-way
                # through the previous layer); front-load three of them so
                # the Q7/SDMA engines have work while AG_B is in flight.
                for pre in range(min(4, len(w_chunks))):
                    issue_a(pre)
                issue_b(0)
                for ci, (c0, cw) in enumerate(w_chunks):
                    g_a, g_ap = ga_tiles[ci], ga_tiles.get(ci - 1)
                    g_b, g_bp = gb_tiles[ci], gb_tiles.get(ci - 1)
                    ga_tiles.pop(ci - 2, None)
                    gb_tiles.pop(ci - 2, None)
                    if ci + 1 < len(w_chunks):
                        issue_b(ci + 1)
                    for wl in range(cw):
                        w_i = c0 + wl
                        n0 = w_i * 128
                        wn = min(128, NPC - n0)
                        ta, tb = T_a_w[w_i], T_b_w[w_i]
                        tw = ta + tb
                        t0 = int(toff[w_i])
                        oh = oh_pool.tile([128, OH_MAX, WIN], f16, name="oh")
                        nc.vector.tensor_tensor(
                            oh[:, :tw, :],
                            iota[:].unsqueeze(1).broadcast_to([128, tw, WIN]),
                            dstloc[:, t0:t0 + tw].unsqueeze(2)
                                .broadcast_to([128, tw, WIN]),
                            mybir.AluOpType.is_equal)
                        ps = ps_agg.tile([HID, WIN], f32, name="psa")
                        for t in range(tw):
                            if t < ta:
                                g = int(offs_a[w_i]) + t
                                if g < F_a[ci]:
                                    lhsT = g_ap[:, g - F_a[ci - 1], 0:HID]
                                else:
                                    lhsT = g_a[:, g - F_a[ci], 0:HID]
                            else:
                                g = int(offs_b[w_i]) + (t - ta)
                                if g < F_b[ci]:
                                    lhsT = g_bp[:, g - F_b[ci - 1], 0:HID]
                                else:
                                    lhsT = g_b[:, g - F_b[ci], 0:HID]
                            nc.tensor.matmul(ps[:], lhsT, oh[:, t, :],
                                             start=(t == 0),
                                             stop=(t == tw - 1))
                        nc.vector.tensor_tensor(
                            h_state[(l + 1) % 2][:, n0:n0 + wn], ps[:, :wn],
                            h_state[l % 2][:, n0:n0 + wn],
                            mybir.AluOpType.add)
                    if ci + 4 < len(w_chunks):
                        issue_a(ci + 4)
                    adv((c0 + cw) * WIN)

    nc.compile()
    return nc


def _get_nc_and_inputs(inputs):
    from concourse import bass_utils  # noqa: F401  (path setup)

    x = np.asarray(inputs["x"], np.float32)
    edge_index = np.asarray(inputs["edge_index"], np.int32)
    W_in = np.asarray(inputs["W_in"], np.float32)
    b_in = np.asarray(inputs["b_in"], np.float32)
    W_layers = np.asarray(inputs["W_layers"], np.float32)
    b_layers = np.asarray(inputs["b_layers"], np.float32)
    W_out = np.asarray(inputs["W_out"], np.float32)
    b_out = np.asarray(inputs["b_out"], np.float32)

    idx_a_w, idx_b_w, dstloc, T_a_w, T_b_w, perm = _prep(edge_index)

    key = ("nc", T_a_w, T_b_w)
    if key not in _cache:
        _cache.clear()
        _cache[key] = _build(T_a_w, T_b_w)
    nc = _cache[key]

    inv = np.empty(N_NODES, np.int64)
    inv[perm] = np.arange(N_NODES)
    xT = np.ascontiguousarray(x.T[:, inv])
    w_inT = np.ascontiguousarray(W_in.T)

    # Linearized GIN: h_{l+1} = (I+A) h_l W^T + b commutes, so
    # out = sum_k g_k C_k + r with g_k = (I+A)^k h0,
    # C_k = (W1^T..Wk^T) Wout_k^T, and r a rank-3 per-node correction
    # from the bias propagation (1, 1+d, 1+2d+Ad directions).
    Wok = [W_out[:, k * HID:(k + 1) * HID].astype(np.float64)
           for k in range(N_LAYERS + 1)]
    Wl = [W_layers[k].astype(np.float64) for k in range(N_LAYERS)]
    bl = [b_layers[k].astype(np.float64) for k in range(N_LAYERS)]
    B = [np.eye(HID)]
    for k in range(N_LAYERS):
        B.append(B[k] @ Wl[k].T)
    C = [B[k] @ Wok[k].T for k in range(N_LAYERS + 1)]
    w_out4 = np.ascontiguousarray(np.stack(C)).astype(np.float16)

    src0 = edge_index[0].astype(np.int64)
    dst0 = edge_index[1].astype(np.int64)
    d = np.bincount(dst0, minlength=N_NODES).astype(np.float64)
    Ad = np.zeros(N_NODES)
    np.add.at(Ad, dst0, d[src0])
    e1 = 1.0 + d
    e2 = 1.0 + 2.0 * d + Ad
    beta1 = (b_out.astype(np.float64) + Wok[1] @ bl[0] + Wok[2] @ bl[1]
             + Wok[3] @ bl[2])
    beta2 = Wok[2] @ Wl[1] @ bl[0] + Wok[3] @ Wl[2] @ bl[1]
    beta3 = Wok[3] @ Wl[2] @ Wl[1] @ bl[0]
    r = (beta1[None, :] + np.outer(e1, beta2)
         + np.outer(e2, beta3))                      # [N, OUT_DIM]
    rT = np.ascontiguousarray(r.T[:, inv]).astype(np.float16)

    iota = np.tile(np.arange(WIN, dtype=np.float16), (128, 1))
    id96 = np.eye(HID, dtype=np.float16)
    id128 = np.eye(128, dtype=np.float32)

    in_maps = []
    for c in range(N_CORES):
        in_maps.append({
            "xT": np.ascontiguousarray(xT[:, c * NPC:(c + 1) * NPC]),
            "w_inT": w_inT,
            "b_in": b_in.reshape(HID, 1),
            "w_out4": w_out4,
            "rcorrT": np.ascontiguousarray(rT[:, c * NPC:(c + 1) * NPC]),
            "iota": iota,
            "id96": id96,
            "id128": id128,
            "idx_a": idx_a_w[c],
            "idx_b": idx_b_w[c],
            "dstloc": dstloc[c],
        })
    return nc, in_maps, perm


def run(inputs, trace=False):
    from concourse import bass_utils

    nc, in_maps, perm = _get_nc_and_inputs(inputs)
    res = bass_utils.run_bass_kernel_spmd(
        nc, in_maps, core_ids=list(range(N_CORES)), trace=trace)
    out = np.concatenate([res.results[c]["out"] for c in range(N_CORES)], 0)
    return out[perm], res


def kernel(**inputs):
    out, _ = run(inputs, trace=False)
    return out



# revision 67
# speedup vs baseline: 1.0382x; 1.0382x over previous
"""GIN-style 3-layer GNN encoder on 8 Trainium2 NeuronCores (Bass/Tile).

Reference computation (fp32):
    h = x @ W_in.T + b_in                                  [50000, 96]
    for l in 0..2:
        agg = segment_sum(h[src], dst, N)                  [50000, 96]
        h = (h + agg) @ W_layers[l].T + b_layers[l]
    out = concat([h0..h3], 1) @ W_out.T + b_out            [50000, 128]

The layers are linear, so the per-layer weight matmuls commute past the
aggregations:  out = sum_k g_k C_k + r  with g_k = (I+A)^k h0,
C_k = (W1^T..Wk^T) Wout_k^T and r a rank-3 per-node bias correction from
(1, 1+d, 1+2d+Ad) — all host-precomputed.  On device each layer is then
just g_{k+1} = g_k + A g_k, which shortens the window -> AllGather path.

Distribution: nodes are partitioned across the 8 cores (6250/core) via a
host-side permutation; each edge is owned by the core that owns its dst
node.  Each layer the updated features are AllGathered into per-layer
replicated row-major fp16 tables h_fullA/h_fullB (Shared DRAM; the A/B
split is window-aligned at 3200 nodes so AG_A fires mid-layer and hides
under the B-half compute, while AG_B's latency is covered by
front-loading the next layer's A-class gathers; 8*HALF keeps gather
indices int16-safe).

Per-core segment sum: a core's node range is split into 49 windows of
128 nodes with per-window tile counts T_a_w/T_b_w (shared across cores;
the permutation greedily minimizes sum_w max_core ceil(count/128), ~7%
padding).  Edge features are fetched with gpsimd dma_gather (fp16 256B
rows, 1024 idxs per instruction = one 64-descriptor packet per SDMA
engine — the hardware maximum — round-robin over the 4 SWDGE queues,
num_idxs registers hoisted).  The Q7 descriptor generation (~1us fixed +
~2ns/idx) is the overall bottleneck, so everything else is scheduled to
hide under it.  For each window the one-hot
onehot[e, t, j] = (dst_local[e, t] == j) is built on DVE with one
broadcast is_equal, and the PE accumulates
    psum[96, 128] += gathered_tile[128e, 96].T @ onehot_tile[128e, 128]
which is aggT for the window; a DVE add writes g_{k+1} = g_k + agg
directly, PE transposes stream the row-major shard, and the two
AllGathers fire as soon as their half of the windows is transposed.
A tiny warm-up collective absorbs the NRT first-collective barrier.
"""
import sys

sys.path.insert(0, "/opt/trn_rl_repo")

import numpy as np

N_NODES = 50000
N_EDGES = 800000
IN_DIM = 128
HID = 96
OUT_DIM = 128
N_LAYERS = 3
N_CORES = 8
NPC = N_NODES // N_CORES          # 6250 nodes per core
WIN = 128                         # window width (nodes)
NW = (NPC + WIN - 1) // WIN       # 49 windows per core (last = 106 nodes)
HALF = 3200                       # per-core A/B split, window-aligned
CLS = N_CORES * HALF              # 31744: A-class size
AW = HALF // WIN                  # 31 full-A windows per core
REM_A = HALF - AW * WIN           # 0: split is window-aligned
CHUNK_W = 4                       # windows per gather buffer
GT = 8                            # tiles per dma_gather (1024 idxs)
CW_N = 512                        # node-chunk for dense matmuls

_cache = {}


def _balance_nodes(src0, dst0):
    """Permute node ids to minimize total gather tiles.

    A node's A/B class (which replicated gather table its row lives in) is
    frozen to its OLD id (< CLS -> A).  The greedy places heavy nodes
    first, charging each placement the number of 128-edge tiles it would
    ADD to the per-window profile T_x[w] = max_c ceil(load_x[c,w]/128),
    so the final per-window tile counts (shared across cores, compile-time
    loop bounds) carry minimal padding.  Returns perm (old id -> new id).
    """
    deg_a = np.bincount(dst0[src0 < CLS], minlength=N_NODES).astype(np.int64)
    deg_b = np.bincount(dst0[src0 >= CLS], minlength=N_NODES).astype(np.int64)
    nbins = N_CORES * NW
    base = np.empty(nbins, np.int64)
    cap = np.empty(nbins, np.int64)
    w_of = np.empty(nbins, np.int64)
    for b in range(nbins):
        c, w = divmod(b, NW)
        base[b] = c * NPC + w * WIN
        cap[b] = min(WIN, NPC - w * WIN)
        w_of[b] = w
    woff = base % NPC
    q_a = np.clip(HALF - woff, 0, cap)   # A slots = first q_a of the window
    q_b = cap - q_a

    mu_a = max(1.0, deg_a.sum() / nbins)
    mu_b = max(1.0, deg_b.sum() / nbins)
    order = np.argsort(-(deg_a + deg_b), kind="stable")
    a_load = np.zeros(nbins, np.int64)
    b_load = np.zeros(nbins, np.int64)
    tmax_a = np.zeros(NW, np.int64)      # current per-window tile profile
    tmax_b = np.zeros(NW, np.int64)
    a_left = q_a.copy()
    b_left = q_b.copy()
    a_pos = np.zeros(nbins, np.int64)
    b_pos = q_a.copy()
    perm = np.empty(N_NODES, np.int64)
    BIG = 1e7
    for n in order:
        na = a_load + deg_a[n]
        nb = b_load + deg_b[n]
        add = (np.maximum(0, -(-na // 128) - tmax_a[w_of])
               + np.maximum(0, -(-nb // 128) - tmax_b[w_of]))
        # the tmax bias spreads profile growth across windows (flat
        # profiles keep per-chunk gather buffers small and uniform)
        phi = (add * BIG + np.maximum(na / mu_a, nb / mu_b)
               + (tmax_a[w_of] + tmax_b[w_of]) * 50.0)
        if n < CLS:
            phi = np.where(a_left > 0, phi, np.inf)
            b_ = int(np.argmin(phi))
            perm[n] = base[b_] + a_pos[b_]
            a_pos[b_] += 1
            a_left[b_] -= 1
        else:
            phi = np.where(b_left > 0, phi, np.inf)
            b_ = int(np.argmin(phi))
            perm[n] = base[b_] + b_pos[b_]
            b_pos[b_] += 1
            b_left[b_] -= 1
        a_load[b_] += deg_a[n]
        b_load[b_] += deg_b[n]
        w_ = w_of[b_]
        tmax_a[w_] = max(tmax_a[w_], -(-a_load[b_] // 128))
        tmax_b[w_] = max(tmax_b[w_], -(-b_load[b_] // 128))
    return perm


def _prep(edge_index):
    """Host-side edge bucketing -> per-core gather index / dst tables.

    Tile counts are per-window (T_a_w, T_b_w — shared across cores), so
    padding is only the across-core ceil gap the balancer leaves."""
    src0 = edge_index[0].astype(np.int64)
    dst0 = edge_index[1].astype(np.int64)
    perm = _balance_nodes(src0, dst0)
    src = perm[src0]
    dst = perm[dst0]
    core = dst // NPC
    din = dst % NPC
    w = din // WIN
    dstl = din % WIN
    s_in = src % NPC
    c_src = src // NPC
    is_b = (s_in >= HALF).astype(np.int64)
    pos = np.where(is_b == 0, c_src * HALF + s_in,
                   c_src * (NPC - HALF) + s_in - HALF)  # int16-safe

    key = (core * NW + w) * 2 + is_b
    order = np.argsort(key, kind="stable")
    s_pos = pos[order]
    s_dstl = dstl[order]
    s_key = key[order]
    s_b = is_b[order]

    counts = np.bincount(key, minlength=N_CORES * NW * 2)
    cnt_cw = counts.reshape(N_CORES, NW, 2)
    T_a_w = np.maximum(1, -(-cnt_cw[:, :, 0].max(axis=0) // 128))  # [NW]
    T_b_w = np.maximum(1, -(-cnt_cw[:, :, 1].max(axis=0) // 128))
    offs_a = np.zeros(NW + 1, np.int64)
    offs_a[1:] = np.cumsum(T_a_w)
    offs_b = np.zeros(NW + 1, np.int64)
    offs_b[1:] = np.cumsum(T_b_w)
    toff = np.zeros(NW + 1, np.int64)
    toff[1:] = np.cumsum(T_a_w + T_b_w)
    TA, TB = int(offs_a[-1]), int(offs_b[-1])
    TT = int(toff[-1])

    starts = np.zeros(N_CORES * NW * 2, np.int64)
    starts[1:] = np.cumsum(counts)[:-1]
    rank = np.arange(len(s_key)) - starts[s_key]

    c_arr = s_key // (2 * NW)
    w_arr = (s_key // 2) % NW

    TA8 = -(-TA // 8) * 8   # fetch ranges are 8-tile aligned
    TB8 = -(-TB // 8) * 8
    idx_a = np.zeros((N_CORES, TA8 * 128), np.int16)
    idx_b = np.zeros((N_CORES, TB8 * 128), np.int16)
    dstl_arr = np.full((N_CORES, TT, 128), -1.0, np.float16)

    a_m = s_b == 0
    flat = c_arr[a_m] * (TA8 * 128) + offs_a[w_arr[a_m]] * 128 + rank[a_m]
    idx_a.reshape(-1)[flat] = s_pos[a_m].astype(np.int16)
    flat = ((c_arr[a_m] * TT + toff[w_arr[a_m]] + rank[a_m] // 128) * 128
            + rank[a_m] % 128)
    dstl_arr.reshape(-1)[flat] = s_dstl[a_m].astype(np.float16)

    b_m = ~a_m
    flat = c_arr[b_m] * (TB8 * 128) + offs_b[w_arr[b_m]] * 128 + rank[b_m]
    idx_b.reshape(-1)[flat] = s_pos[b_m].astype(np.int16)
    t_g = T_a_w[w_arr[b_m]] + rank[b_m] // 128
    flat = (c_arr[b_m] * TT + toff[w_arr[b_m]] + t_g) * 128 + rank[b_m] % 128
    dstl_arr.reshape(-1)[flat] = s_dstl[b_m].astype(np.float16)

    def wrap(vals):  # [T*128] -> [128, T*8] int16 wrapped+replicated
        v = vals.reshape(-1, 16).T
        return np.tile(v, (8, 1)).copy()

    idx_a_w = np.stack([wrap(idx_a[c]) for c in range(N_CORES)])
    idx_b_w = np.stack([wrap(idx_b[c]) for c in range(N_CORES)])
    dstloc = np.ascontiguousarray(dstl_arr.transpose(0, 2, 1))  # [C,128,TT]
    return (idx_a_w, idx_b_w, dstloc, tuple(int(t) for t in T_a_w),
            tuple(int(t) for t in T_b_w), perm)


def _build(T_a_w, T_b_w):
    from concourse import bacc, tile, mybir, library_config

    dt = mybir.dt
    offs_a = np.zeros(NW + 1, np.int64)
    offs_a[1:] = np.cumsum(T_a_w)
    offs_b = np.zeros(NW + 1, np.int64)
    offs_b[1:] = np.cumsum(T_b_w)
    toff = np.zeros(NW + 1, np.int64)
    toff[1:] = np.cumsum(np.add(T_a_w, T_b_w))
    TA, TB, TT = int(offs_a[-1]), int(offs_b[-1]), int(toff[-1])
    TA8 = -(-TA // 8) * 8
    TB8 = -(-TB // 8) * 8
    nc = bacc.Bacc("TRN2", target_bir_lowering=False, debug=False,
                   num_devices=N_CORES, num_swdge_queues=4)

    # ---- I/O ----
    xT_in = nc.dram_tensor("xT", [IN_DIM, NPC], dt.float32, kind="ExternalInput")
    w_inT_in = nc.dram_tensor("w_inT", [IN_DIM, HID], dt.float32,
                              kind="ExternalInput")
    b_in_in = nc.dram_tensor("b_in", [HID, 1], dt.float32, kind="ExternalInput")
    w_out4_in = nc.dram_tensor("w_out4", [N_LAYERS + 1, HID, OUT_DIM],
                               dt.float16, kind="ExternalInput")
    rcorrT_in = nc.dram_tensor("rcorrT", [OUT_DIM, NPC], dt.float16,
                               kind="ExternalInput")
    iota_in = nc.dram_tensor("iota", [128, WIN], dt.float16,
                             kind="ExternalInput")
    id96_in = nc.dram_tensor("id96", [HID, HID], dt.float16,
                             kind="ExternalInput")
    id128_in = nc.dram_tensor("id128", [128, 128], dt.float32,
                              kind="ExternalInput")
    idx_a_in = nc.dram_tensor("idx_a", [128, TA8 * 8], dt.int16,
                              kind="ExternalInput")
    idx_b_in = nc.dram_tensor("idx_b", [128, TB8 * 8], dt.int16,
                              kind="ExternalInput")
    dstloc_in = nc.dram_tensor("dstloc", [128, TT], dt.float16,
                               kind="ExternalInput")
    out_ext = nc.dram_tensor("out", [NPC, OUT_DIM], dt.float32,
                             kind="ExternalOutput")

    f32, f32r, f16 = dt.float32, dt.float32r, dt.float16

    with tile.TileContext(nc, num_cores=N_CORES) as tc:
        nc.gpsimd.load_library(library_config.mlp)
        with tc.tile_pool(name="persist", bufs=1) as pp, \
             tc.tile_pool(name="xpool", bufs=5) as xpool, \
             tc.tile_pool(name="ga", bufs=4) as ga_pool, \
             tc.tile_pool(name="gb", bufs=3) as gb_pool, \
             tc.tile_pool(name="oh", bufs=2) as oh_pool, \
             tc.tile_pool(name="otile", bufs=2) as ot_pool, \
             tc.tile_pool(name="ps_agg", bufs=4, space="PSUM") as ps_agg, \
             tc.tile_pool(name="ps_big", bufs=2, space="PSUM") as ps_big, \
             tc.tile_pool(name="ps_tr", bufs=2, space="PSUM") as ps_tr, \
             tc.tile_pool(name="dram", bufs=1, space="DRAM") as dram:

            def load(name, shape, dtype, src_ap):
                t = pp.tile(shape, dtype, name=name)
                nc.sync.dma_start(out=t[:], in_=src_ap)
                return t

            w_inT = load("w_inT", [IN_DIM, HID], f32r, w_inT_in[:].bitcast(f32r))
            b_in = load("b_in", [HID, 1], f32, b_in_in[:])
            iota = load("iota", [128, WIN], f16, iota_in[:])
            id96 = load("id96", [HID, HID], f16, id96_in[:])
            id128 = load("id128", [128, 128], f32, id128_in[:])

            h_state = [pp.tile([HID, NPC], f16, name=f"h{s}")
                       for s in range(N_LAYERS + 1)]
            rm_buf = pp.tile([128, NW, 128], f16, name="rm_buf")

            # One table pair per layer: Shared DRAM requires a single
            # writer, and separate tables remove WAR hazards between a
            # layer's gathers and the next AllGather.
            h_fullA = [dram.tile([CLS, 128], f16, addr_space="Shared",
                                 name=f"h_fullA{s}")
                       for s in range(N_LAYERS)]
            h_fullB = [dram.tile([N_NODES - CLS, 128], f16,
                                 addr_space="Shared", name=f"h_fullB{s}")
                       for s in range(N_LAYERS)]
            bounceA = dram.tile([HALF, 128], f16)
            bounceB = dram.tile([NPC - HALF, 128], f16)

            node_chunks = [(j * CW_N, min(CW_N, NPC - j * CW_N))
                           for j in range(-(-NPC // CW_N))]

            def transpose_windows(s, w0, w1):
                for t in range(w0, w1):
                    n0 = t * 128
                    tn = min(128, NPC - n0)
                    pst = ps_tr.tile([128, HID], f16, name="pst")
                    nc.tensor.transpose(pst[:tn, :],
                                        h_state[s][:, n0:n0 + tn], id96[:])
                    nc.scalar.copy(rm_buf[:tn, t, 0:HID], pst[:tn, :])

            def bounce_ag_a(s):
                nc.sync.dma_start(
                    out=bounceA[0:AW * 128, :].rearrange(
                        "(t p) d -> p t d", p=128),
                    in_=rm_buf[:, 0:AW, :])
                nc.gpsimd.collective_compute(
                    "AllGather", mybir.AluOpType.bypass,
                    ins=[bounceA.opt()], outs=[h_fullA[s].opt()],
                    replica_groups=[list(range(N_CORES))])

            def bounce_ag_b(s):
                nb_full = NW - AW - 1   # full windows AW .. NW-2
                nc.sync.dma_start(
                    out=bounceB[0:nb_full * 128, :].rearrange(
                        "(t p) d -> p t d", p=128),
                    in_=rm_buf[:, AW:NW - 1, :])
                o1 = nb_full * 128
                last_n = NPC - (NW - 1) * WIN
                nc.sync.dma_start(out=bounceB[o1:o1 + last_n, :],
                                  in_=rm_buf[0:last_n, NW - 1, :])
                nc.gpsimd.collective_compute(
                    "AllGather", mybir.AluOpType.bypass,
                    ins=[bounceB.opt()], outs=[h_fullB[s].opt()],
                    replica_groups=[list(range(N_CORES))])

            def out_proj(n0, cw):
                ps = ps_big.tile([OUT_DIM, CW_N], f32, name="pso", tag="psb")
                for s4 in range(N_LAYERS + 1):
                    nc.tensor.matmul(ps[:, :cw], w_out4[s4][:],
                                     h_state[s4][:, n0:n0 + cw],
                                     start=(s4 == 0), stop=(s4 == N_LAYERS))
                ot = ot_pool.tile([OUT_DIM, CW_N], f32, name="ot")
                nc.vector.tensor_tensor(ot[:, :cw], ps[:, :cw],
                                        rcorrT[:, n0:n0 + cw],
                                        mybir.AluOpType.add)
                for tt in range(-(-cw // 128)):
                    t0 = tt * 128
                    tn = min(128, cw - t0)
                    pst = ps_tr.tile([128, 128], f32, name="psto", tag="pst")
                    nc.tensor.transpose(pst[:tn, :], ot[:, t0:t0 + tn],
                                        id128[:])
                    orow = ot_pool.tile([128, 128], f32, name="orow")
                    nc.scalar.copy(orow[:tn, :], pst[:tn, :])
                    nc.sync.dma_start(
                        out=out_ext[n0 + t0:n0 + t0 + tn, :],
                        in_=orow[:tn, :])

            PH_A_END = AW * WIN   # 3968: phase-A node frontier

            def make_advance(s):
                """Incrementally issue dense matmul / transposes / AGs for
                h_state[s] as the node frontier moves.  AG_A fires as soon
                as window AW is transposed (mid-layer), AG_B at the end, so
                each collective overlaps the other half's compute."""
                st = {"dense": 0, "trans": 0}

                def advance(frontier):
                    frontier = min(frontier, NPC)
                    if s == 0 or s == N_LAYERS:
                        while st["dense"] < frontier:
                            n0 = st["dense"]
                            lim = PH_A_END if n0 < PH_A_END else NPC
                            cw = min(CW_N, frontier - n0, lim - n0)
                            if s == 0:
                                xb = xpool.tile([IN_DIM, CW_N], f32r,
                                                name="xb")
                                nc.sync.dma_start(
                                    out=xb[:, :cw],
                                    in_=xT_in[:, n0:n0 + cw].bitcast(f32r))
                                ps = ps_big.tile([HID, CW_N], f32, name="psb")
                                nc.tensor.matmul(ps[:, :cw], w_inT[:],
                                                 xb[:, :cw],
                                                 start=True, stop=True)
                                nc.scalar.add(h_state[0][:, n0:n0 + cw],
                                              ps[:, :cw], b_in[:])
                            else:
                                out_proj(n0, cw)
                            st["dense"] += cw
                    else:
                        # h_state[s] is written directly by the window
                        # aggregation adds; just move the frontier.
                        st["dense"] = max(st["dense"], frontier)
                    if s < N_LAYERS:
                        while st["trans"] < NW and \
                                min((st["trans"] + 1) * WIN, NPC) <= st["dense"]:
                            transpose_windows(s, st["trans"], st["trans"] + 1)
                            w_done = st["trans"]
                            st["trans"] += 1
                            if w_done == AW - 1:
                                bounce_ag_a(s)
                            elif w_done == NW - 1:
                                bounce_ag_b(s)
                return advance

            qrr = [0]
            nreg = {}   # hoisted num_idxs registers (avoid a MOVE per gather)

            def emit_gathers(gbuf, src_view, idx_tile, base_tile, n_tiles):
                for s0 in range(0, n_tiles, GT):
                    sn = min(GT, n_tiles - s0)
                    v = sn * 128
                    if v not in nreg:
                        nreg[v] = nc.gpsimd.to_reg(v)
                    nc.gpsimd.dma_gather(
                        gbuf[:, s0:s0 + sn, :], src_view,
                        idx_tile[:, (base_tile + s0) * 8:
                                 (base_tile + s0 + sn) * 8],
                        num_idxs=v, num_idxs_reg=nreg[v],
                        elem_size=128, single_packet=True,
                        queue_num=qrr[0] % 4)
                    qrr[0] += 1

            # Tiny warm-up collective: absorbs the NRT first-collective
            # barrier (~30us) under the input loads / projection.  (A
            # dependency-free variant that dispatches immediately measured
            # consistently WORSE — the barrier rendezvous runs long when
            # entered before the cores finish setup — so the short
            # memset->DMA chain before dispatch is deliberate.)
            warm_src = dram.tile([1, 128], f16)
            warm_dst = dram.tile([N_CORES, 128], f16, addr_space="Shared")
            warm_sb = pp.tile([1, 128], f16, name="warm_sb")
            nc.vector.memset(warm_sb[:], 0.0)
            nc.sync.dma_start(out=warm_src[:], in_=warm_sb[:])
            nc.gpsimd.collective_compute(
                "AllGather", mybir.AluOpType.bypass,
                ins=[warm_src.opt()], outs=[warm_dst.opt()],
                replica_groups=[list(range(N_CORES))])

            # ---- input projection (phase-split epilogue) ----
            adv0 = make_advance(0)
            adv0(PH_A_END)
            # bulk tables load after the phase-A projection's x chunks so
            # the first AllGather isn't queued behind them
            idx_a = load("idx_a", [128, TA8 * 8], dt.int16, idx_a_in[:])
            idx_b = load("idx_b", [128, TB8 * 8], dt.int16, idx_b_in[:])
            dstloc = load("dstloc", [128, TT], f16, dstloc_in[:])
            w_out4 = [load(f"w_out4_{s}", [HID, OUT_DIM], f16, w_out4_in[s])
                      for s in range(N_LAYERS + 1)]
            rcorrT = load("rcorrT", [OUT_DIM, NPC], f16, rcorrT_in[:])
            adv0(NPC)

            # ---- GIN layers ----
            w_chunks = [(c0, min(CHUNK_W, NW - c0))
                        for c0 in range(0, NW, CHUNK_W)]
            # Instruction-aligned fetch ranges: chunk buffer c fetches
            # global tiles [F[c], F[c+1]), F multiples of 8 tiles, so every
            # dma_gather is a full 1024-idx instruction (the tail spills a
            # few of the next chunk's tiles into this buffer; boundary
            # windows read from two buffers).
            def fetch_bounds(offs):
                F = [0]
                for c0, cw in w_chunks:
                    F.append(int(-(-int(offs[c0 + cw]) // 8) * 8))
                return F
            F_a = fetch_bounds(offs_a)
            F_b = fetch_bounds(offs_b)
            GA_MAX = max(F_a[i + 1] - F_a[i] for i in range(len(w_chunks)))
            GB_MAX = max(F_b[i + 1] - F_b[i] for i in range(len(w_chunks)))
            OH_MAX = max(T_a_w[w] + T_b_w[w] for w in range(NW))
            for l in range(N_LAYERS):
                adv = make_advance(l + 1)
                ga_tiles = {}
                gb_tiles = {}

                def issue_a(ci, l=l):
                    nt = F_a[ci + 1] - F_a[ci]
                    g = ga_pool.tile([128, GA_MAX, 128], f16, name="g_a")
                    emit_gathers(g, h_fullA[l][:], idx_a, F_a[ci], nt)
                    ga_tiles[ci] = g

                def issue_b(ci, l=l):
                    nt = F_b[ci + 1] - F_b[ci]
                    g = gb_pool.tile([128, GB_MAX, 128], f16, name="g_b")
                    emit_gathers(g, h_fullB[l][:], idx_b, F_b[ci], nt)
                    gb_tiles[ci] = g

                # A-class gathers only need AG_A (already landed mid-way
                # through the previous layer); front-load three of them so
                # the Q7/SDMA engines have work while AG_B is in flight.
                for pre in range(min(3, len(w_chunks))):
                    issue_a(pre)
                issue_b(0)
                for ci, (c0, cw) in enumerate(w_chunks):
                    g_a, g_ap = ga_tiles[ci], ga_tiles.get(ci - 1)
                    g_b, g_bp = gb_tiles[ci], gb_tiles.get(ci - 1)
                    ga_tiles.pop(ci - 2, None)
                    gb_tiles.pop(ci - 2, None)
                    if ci + 1 < len(w_chunks):
                        issue_b(ci + 1)
                    for wl in range(cw):
                        w_i = c0 + wl
                        n0 = w_i * 128
                        wn = min(128, NPC - n0)
                        ta, tb = T_a_w[w_i], T_b_w[w_i]
                        tw = ta + tb
                        t0 = int(toff[w_i])
                        oh = oh_pool.tile([128, OH_MAX, WIN], f16, name="oh")
                        nc.vector.tensor_tensor(
                            oh[:, :tw, :],
                            iota[:].unsqueeze(1).broadcast_to([128, tw, WIN]),
                            dstloc[:, t0:t0 + tw].unsqueeze(2)
                                .broadcast_to([128, tw, WIN]),
                            mybir.AluOpType.is_equal)
                        ps = ps_agg.tile([HID, WIN], f32, name="psa")
                        for t in range(tw):
                            if t < ta:
                                g = int(offs_a[w_i]) + t
                                if g < F_a[ci]:
                                    lhsT = g_ap[:, g - F_a[ci - 1], 0:HID]
                                else:
                                    lhsT = g_a[:, g - F_a[ci], 0:HID]
                            else:
                                g = int(offs_b[w_i]) + (t - ta)
                                if g < F_b[ci]:
                                    lhsT = g_bp[:, g - F_b[ci - 1], 0:HID]
                                else:
                                    lhsT = g_b[:, g - F_b[ci], 0:HID]
                            nc.tensor.matmul(ps[:], lhsT, oh[:, t, :],
                                             start=(t == 0),
                                             stop=(t == tw - 1))
                        nc.vector.tensor_tensor(
                            h_state[l + 1][:, n0:n0 + wn], ps[:, :wn],
                            h_state[l][:, n0:n0 + wn], mybir.AluOpType.add)
                    if ci + 3 < len(w_chunks):
                        issue_a(ci + 3)
                    adv((c0 + cw) * WIN)

    nc.compile()
    return nc


def _get_nc_and_inputs(inputs):
    from concourse import bass_utils  # noqa: F401  (path setup)

    x = np.asarray(inputs["x"], np.float32)
    edge_index = np.asarray(inputs["edge_index"], np.int32)
    W_in = np.asarray(inputs["W_in"], np.float32)
    b_in = np.asarray(inputs["b_in"], np.float32)
    W_layers = np.asarray(inputs["W_layers"], np.float32)
    b_layers = np.asarray(inputs["b_layers"], np.float32)
    W_out = np.asarray(inputs["W_out"], np.float32)
    b_out = np.asarray(inputs["b_out"], np.float32)

    idx_a_w, idx_b_w, dstloc, T_a_w, T_b_w, perm = _prep(edge_index)

    key = ("nc", T_a_w, T_b_w)
    if key not in _cache:
        _cache.clear()
        _cache[key] = _build(T_a_w, T_b_w)
    nc = _cache[key]

    inv = np.empty(N_NODES, np.int64)
    inv[perm] = np.arange(N_NODES)
    xT = np.ascontiguousarray(x.T[:, inv])
    w_inT = np.ascontiguousarray(W_in.T)

    # Linearized GIN: h_{l+1} = (I+A) h_l W^T + b commutes, so
    # out = sum_k g_k C_k + r with g_k = (I+A)^k h0,
    # C_k = (W1^T..Wk^T) Wout_k^T, and r a rank-3 per-node correction
    # from the bias propagation (1, 1+d, 1+2d+Ad directions).
    Wok = [W_out[:, k * HID:(k + 1) * HID].astype(np.float64)
           for k in range(N_LAYERS + 1)]
    Wl = [W_layers[k].astype(np.float64) for k in range(N_LAYERS)]
    bl = [b_layers[k].astype(np.float64) for k in range(N_LAYERS)]
    B = [np.eye(HID)]
    for k in range(N_LAYERS):
        B.append(B[k] @ Wl[k].T)
    C = [B[k] @ Wok[k].T for k in range(N_LAYERS + 1)]
    w_out4 = np.ascontiguousarray(np.stack(C)).astype(np.float16)

    src0 = edge_index[0].astype(np.int64)
    dst0 = edge_index[1].astype(np.int64)
    d = np.bincount(dst0, minlength=N_NODES).astype(np.float64)
    Ad = np.zeros(N_NODES)
    np.add.at(Ad, dst0, d[src0])
    e1 = 1.0 + d
    e2 = 1.0 + 2.0 * d + Ad
    beta1 = (b_out.astype(np.float64) + Wok[1] @ bl[0] + Wok[2] @ bl[1]
             + Wok[3] @ bl[2])
    beta2 = Wok[2] @ Wl[1] @ bl[0] + Wok[3] @ Wl[2] @ bl[1]
    beta3 = Wok[3] @ Wl[2] @ Wl[1] @ bl[0]
    r = (beta1[None, :] + np.outer(e1, beta2)
         + np.outer(e2, beta3))                      # [N, OUT_DIM]
    rT = np.ascontiguousarray(r.T[:, inv]).astype(np.float16)

    iota = np.tile(np.arange(WIN, dtype=np.float16), (128, 1))
    id96 = np.eye(HID, dtype=np.float16)
    id128 = np.eye(128, dtype=np.float32)

    in_maps = []
    for c in range(N_CORES):
        in_maps.append({
            "xT": np.ascontiguousarray(xT[:, c * NPC:(c + 1) * NPC]),
            "w_inT": w_inT,
            "b_in": b_in.reshape(HID, 1),
            "w_out4": w_out4,
            "rcorrT": np.ascontiguousarray(rT[:, c * NPC:(c + 1) * NPC]),
            "iota": iota,
            "id96": id96,
            "id128": id128,
            "idx_a": idx_a_w[c],
            "idx_b": idx_b_w[c],
            "dstloc": dstloc[c],
        })
    return nc, in_maps, perm


def run(inputs, trace=False):
    from concourse import bass_utils

    nc, in_maps, perm = _get_nc_and_inputs(inputs)
    res = bass_utils.run_bass_kernel_spmd(
        nc, in_maps, core_ids=list(range(N_CORES)), trace=trace)
    out = np.concatenate([res.results[c]["out"] for c in range(N_CORES)], 0)
    return out[perm], res


def kernel(**inputs):
    out, _ = run(inputs, trace=False)
    return out

